# revision 53
# baseline (speedup 1.0000x reference)
"""Distributed Bass kernel for nn_LACF (gnn_message_passing) on 8 TRN2 cores.

Strategy: shard nodes (and their incoming edges, via h_idx) across 8 cores.
Each core owns RS=N/8 rows. Edges are bucketed by (core, 128-node block,
t-class) where t-class = one of NC_WIN=4 windows of the global packed-table
row space (each window < 32768 rows so dma_gather's int16 indices reach it).

Per layer (interleaved per 4-block group g):
  edge_group(i, g): for each class c: one batched dma_gather of packed
    512B rows (pfull window c) + one dma_gather of A1[h] rows (f32, local)
    + edge MLP on the whole chunk + fused 193-col segsum matmuls
    (PSUM per block, SBUF gacc accumulation across classes).
  node_group(i+1, g): update e/s tables from gacc (SBUF handoff), compute
    A1/B1/x2 for the next layer, write packed bf16 rows into pshard chunk.
AllGather is split into 5 chunked collectives fired as their node groups
complete, overlapping communication with compute.
"""

import sys

if "/opt/trn_rl_repo" not in sys.path:
    sys.path.insert(0, "/opt/trn_rl_repo")

import numpy as np
import ml_dtypes

import os

BF16 = ml_dtypes.bfloat16
G_EPS = np.float32(1e-6)
ROW_EPS = 1e-30
NC_WIN = 4          # t-class windows (each < 32768 rows for int16 idx)
NB_G = 4            # blocks per group (node chunk == edge group)
N_CC = 5            # collective chunks per layer
_KB = os.environ.get("KB_GATHER", "both")   # both | gt | at | none
if os.environ.get("KB_INDIRECT", "") == "1":
    _KB = "none"
GATHER_GT = _KB in ("both", "gt")
GATHER_AT = _KB in ("both", "at")
USE_DMA_GATHER = True                        # idx tensors always built


def _prep(inputs, ncores):
    """Host-side sharding: bucket edges by (core, block, t-class), build
    int16 gather indices, per-slot gumbel/recipG, valued one-hot P tiles."""
    h = np.asarray(inputs["h_idx"]).astype(np.int64).ravel()
    t = np.asarray(inputs["t_idx"]).astype(np.int64).ravel()
    G = np.asarray(inputs["G_values"]).astype(np.float32).ravel()
    eg = np.asarray(inputs["edge_gumbel"]).astype(np.float32)
    emb0 = np.asarray(inputs["emb0"]).astype(np.float32)
    ngum = np.asarray(inputs["emb_gumbel"]).astype(np.float32)

    N, D = emb0.shape
    E = h.shape[0]
    L = eg.shape[0]
    assert N % ncores == 0
    RS = N // ncores                      # real rows per core
    nb = (RS + 127) // 128                # node blocks per core
    R = nb * 128                          # padded rows per core
    NF = ncores * R
    W = -(-NF // NC_WIN)                  # class window rows
    assert W <= 32767

    n_grp = -(-nb // NB_G)
    cs_g = np.minimum(NB_G, nb - NB_G * np.arange(n_grp))  # blocks per group
    gpc = -(-n_grp // N_CC)
    cc_groups = [list(range(k * gpc, min((k + 1) * gpc, n_grp)))
                 for k in range(N_CC)]
    cc_rows = [int(cs_g[gs].sum()) * 128 for gs in cc_groups]
    cc_base = [int(x) for x in
               np.concatenate([[0], np.cumsum(cc_rows)])[:N_CC]]

    # ---- pfull row layout: collective-chunk-major (chunk k holds all 8
    # cores' rows [cc_base[k], +cc_rows[k]) contiguously, so each chunked
    # AllGather writes a contiguous range). Overlapping class windows:
    # window c covers rows [c*W, c*W+32767]; an edge whose quarter-offset
    # is < 32768-W may be assigned to class c-1, letting per-bucket caps
    # like [3,2,2,2] tiles hold.
    core_t = t // RS
    tr = t - core_t * RS
    grp_t = tr // (NB_G * 128)
    k_t = np.minimum(grp_t // gpc, N_CC - 1)
    S_k = np.array(cc_rows, np.int64)
    B_k = np.array(cc_base, np.int64)
    tgid = B_k[k_t] * ncores + core_t * S_k[k_t] + (tr - B_k[k_t])

    core_of = h // RS
    hloc = h - core_of * RS
    blk = hloc // 128
    g_of = blk // NB_G
    bb = blk % NB_G

    q = tgid // W                          # base quarter 0..3
    off = tgid - q * W
    movable = (off < 32768 - W) & (q > 0)
    bucket = core_of * nb + blk
    nbuck = ncores * nb

    nq = np.zeros((nbuck, NC_WIN), np.int64)
    np.add.at(nq, (bucket, q), 1)
    mq = np.zeros((nbuck, NC_WIN), np.int64)
    np.add.at(mq, (bucket, q), movable.astype(np.int64))
    # bottom-up shed: class c must move shed_c movable edges down one class
    profiles = [[3, 2, 2, 2], [3, 3, 2, 2], [3, 2, 2, 3], [3, 3, 3, 2],
                [3, 3, 2, 3], [3, 2, 3, 3], [3, 3, 3, 3], [4, 3, 3, 3],
                [4, 4, 4, 4]]
    for T_cs in profiles:
        caps = np.array([c * 128 for c in T_cs])
        shed = np.zeros((nbuck, NC_WIN + 1), np.int64)
        ok = np.ones(nbuck, bool)
        for c in range(NC_WIN - 1, 0, -1):
            shed[:, c] = np.maximum(nq[:, c] + shed[:, c + 1] - caps[c], 0)
            ok &= shed[:, c] <= mq[:, c]
        ok &= (nq[:, 0] + shed[:, 1]) <= caps[0]
        if ok.all():
            break
    else:
        raise RuntimeError("no feasible class profile")
    kmove = shed[:, :NC_WIN]
    # move the k smallest-t movable edges of each (bucket, q) down one class
    okey = (bucket * NC_WIN + q) * np.int64(NF) + tgid
    order_t = np.argsort(okey, kind="stable")
    seg = (bucket * NC_WIN + q)[order_t]
    seg_start = np.zeros(nbuck * NC_WIN, np.int64)
    cnts = np.bincount(seg, minlength=nbuck * NC_WIN)
    seg_start[1:] = np.cumsum(cnts)[:-1]
    rank_t = np.arange(E) - seg_start[seg]
    moved_sorted = rank_t < kmove.reshape(-1)[seg]
    cls = q.copy()
    cls[order_t[moved_sorted]] -= 1
    idxv = (tgid - cls * W).astype(np.int16)
    assert (tgid - cls * W < 32768).all() and (tgid - cls * W >= 0).all()

    coff = np.concatenate([[0], np.cumsum(T_cs)])[:NC_WIN]  # tile offs
    TPB = int(sum(T_cs))                    # tiles per block
    tiles_total = TPB * nb
    total_slots = tiles_total * 128

    key = (core_of * nb + blk) * NC_WIN + cls
    order = np.argsort(key, kind="stable")
    counts = np.bincount(key, minlength=ncores * nb * NC_WIN)
    starts = np.zeros(ncores * nb * NC_WIN, np.int64)
    starts[1:] = np.cumsum(counts)[:-1]
    sk = key[order]
    rank = np.arange(E) - starts[sk]
    j = rank // 128
    p = rank % 128
    co = core_of[order]
    clso = cls[order]
    go = g_of[order]
    bbo = bb[order]
    csel = np.minimum(NB_G, nb - NB_G * go)
    coff_a = np.asarray(coff)
    T_cs_a = np.asarray(T_cs)
    tile = (TPB * NB_G * go
            + coff_a[clso] * csel + bbo * T_cs_a[clso] + j)
    slot = tile * 128 + p

    ncols = total_slots // 16
    tid16 = np.zeros((ncores, 16, ncols), np.int16)
    egc = np.zeros((ncores, L, 128, tiles_total), np.float32)
    rg = np.zeros((ncores, 128, tiles_total), np.float32)
    p0 = np.zeros((ncores, 128, tiles_total * 128), BF16)
    p1t = np.zeros((ncores, 128, tiles_total * 128), BF16)

    tid16[co, slot % 16, slot // 16] = idxv[order]
    tid32 = np.zeros((ncores, 128, tiles_total), np.int32)
    tid32[co, p, tile] = tgid[order].astype(np.int32)
    egc[co, :, p, tile] = eg[:, order].T
    gsafe = np.maximum(G[order], G_EPS)
    rg[co, p, tile] = np.float32(1.0) / gsafe
    noff = (hloc[order] % 128).astype(np.int64)
    p0[co, p, tile * 128 + noff] = gsafe.astype(BF16)
    p1t[co, noff, tile * 128 + p] = np.ones(E, BF16)

    tid16 = np.tile(tid16, (1, 8, 1))

    embc = np.zeros((ncores, R, D), np.float32)
    gumc = np.zeros((ncores, L, R, D), np.float32)
    for cc in range(ncores):
        embc[cc, :RS] = emb0[cc * RS:(cc + 1) * RS]
        gumc[cc, :, :RS] = ngum[:, cc * RS:(cc + 1) * RS]

    return dict(N=N, D=D, E=E, L=L, RS=RS, nb=nb, R=R, NF=NF, W=W,
                T_cs=[int(x) for x in T_cs], TPB=TPB,
                tiles_total=tiles_total, n_grp=n_grp,
                cs_g=[int(x) for x in cs_g], cc_groups=cc_groups,
                cc_rows=cc_rows, cc_base=cc_base,
                tid16=tid16, tid32=tid32,
                egc=egc, rg=rg, p0=p0, p1t=p1t, embc=embc, gumc=gumc)


def build_program(cfg):
    import concourse.bacc as bacc
    import concourse.mybir as mybir
    import concourse.tile as tile
    from concourse.masks import make_identity

    nb, L, NCC = cfg["nb"], cfg["L"], cfg["ncores"]
    T_cs, TPB = cfg["T_cs"], cfg["TPB"]
    coff = [0]
    for tcv in T_cs[:-1]:
        coff.append(coff[-1] + tcv)
    D = cfg["D"]
    R = nb * 128
    NF = NCC * R
    W = cfg["W"]
    tiles_total = cfg["tiles_total"]
    total_cols = tiles_total * 8          # int16 idx cols
    n_grp = cfg["n_grp"]
    cs_g = cfg["cs_g"]
    cc_groups = cfg["cc_groups"]
    cc_rows = cfg["cc_rows"]
    cc_base = cfg["cc_base"]
    PK = 4 * D
    KT_MAX = NB_G * max(T_cs)
    b2v = cfg["b2"]
    inv_t = cfg["inv_t"]

    f32 = mybir.dt.float32
    bf = mybir.dt.bfloat16
    i16 = mybir.dt.int16

    nc = bacc.Bacc("TRN2", target_bir_lowering=False)

    P_in = {}
    for name, shape, dt in [
        ("emb", [R, D], f32), ("gum", [L, R, D], f32),
        ("p0", [128, tiles_total * 128], bf),
        ("p1t", [128, tiles_total * 128], bf),
        ("tid16", [128, total_cols], i16),
        ("tid32", [128, tiles_total], mybir.dt.int32),
        ("egum", [L, 128, tiles_total], f32),
        ("rgs", [128, tiles_total], f32),
        ("w1t", [L, D, D], f32), ("w1b", [L, D, D], f32), ("b1", [L, D], f32),
        ("w2", [L, 128, KT_MAX * D], f32),
        ("ew1", [L, D, D], f32), ("ew2", [L, D, D], f32),
        ("eb1", [L, D], f32), ("eb2", [L, D], f32),
    ]:
        P_in[name] = nc.dram_tensor(name, shape, dt, kind="ExternalInput")
    out = nc.dram_tensor("out", [3, R, D], f32, kind="ExternalOutput")

    rg_all = [list(range(NCC))]

    with tile.TileContext(nc) as tc:
        with (
            tc.tile_pool(name="dram", bufs=1, space="DRAM") as dram,
            tc.tile_pool(name="const", bufs=1) as constp,
            tc.tile_pool(name="nodew", bufs=2) as nodew,
            tc.tile_pool(name="chunkw", bufs=2) as chunkw,
            tc.tile_pool(name="edgew", bufs=2) as edgew,
            tc.tile_pool(name="gaccp", bufs=3) as gaccp,
            tc.tile_pool(name="ps", bufs=2, space="PSUM") as psp,
            tc.tile_pool(name="psacc", bufs=2, space="PSUM") as psaccp,
        ):
            # ---- persistent DRAM state
            e0d = dram.tile([R, D], f32, name="e0d")
            e1d = dram.tile([R, D], f32, name="e1d")
            e2d = dram.tile([R, D], f32, name="e2d")
            s0d = dram.tile([R, D], f32, name="s0d")
            s1d = dram.tile([R, D], f32, name="s1d")
            s2d = dram.tile([R, D], f32, name="s2d")
            pshard = [dram.tile([cc_rows[k], PK], bf, name=f"pshard{k}")
                      for k in range(N_CC)]
            pfull = [dram.tile([NF, PK], bf, name=f"pfull{i}")
                     for i in range(L)]

            # ---- constants resident in SBUF
            ident = constp.tile([128, 128], f32, name="ident")
            make_identity(nc, ident[:])
            if GATHER_GT:
                tsb = constp.tile([128, total_cols], i16, name="tsb")
                nc.sync.dma_start(out=tsb[:], in_=P_in["tid16"][:, :])
            else:
                tsb32 = constp.tile([128, tiles_total], mybir.dt.int32,
                                    name="tsb32")
                nc.sync.dma_start(out=tsb32[:], in_=P_in["tid32"][:, :])
            # per-block A1 rows, resident in SBUF (written by node phase,
            # consumed by the same group's edge phase next layer)
            a1sb = constp.tile([128, nb, D], bf, name="a1sb")
            rgsb = constp.tile([128, tiles_total], f32, name="rgsb")
            nc.sync.dma_start(out=rgsb[:], in_=P_in["rgs"][:, :])
            egsb = [constp.tile([128, tiles_total], f32, name=f"egsb{i}")
                    for i in range(L)]
            for i in range(L):
                nc.sync.dma_start(out=egsb[i][:], in_=P_in["egum"][i, :, :])
            w2sb = [constp.tile([128, KT_MAX, D], f32, name=f"w2sb{i}")
                    for i in range(L)]
            for i in range(L):
                nc.sync.dma_start(out=w2sb[i][:], in_=P_in["w2"][i, :, :])
            wt = {}
            for wname in ("w1t", "w1b", "ew1", "ew2"):
                for i in range(L):
                    wtile = constp.tile([D, D], f32, name=f"{wname}{i}")
                    nc.sync.dma_start(out=wtile[:], in_=P_in[wname][i, :, :])
                    wt[(wname, i)] = wtile
            for bname in ("b1", "eb1", "eb2"):
                for i in range(L):
                    btile = constp.tile([D, 1], f32, name=f"{bname}{i}")
                    nc.sync.dma_start(out=btile[:], in_=P_in[bname][i, :, None])
                    wt[(bname, i)] = btile

            # ---- prologue: init tables from emb
            for dst in (e0d, e1d, e2d, s0d, s1d, s2d):
                nc.sync.dma_start(out=dst[:], in_=P_in["emb"][:, :])

            Relu = mybir.ActivationFunctionType.Relu
            Sigm = mybir.ActivationFunctionType.Sigmoid
            Ident = mybir.ActivationFunctionType.Identity
            Copy = mybir.ActivationFunctionType.Copy
            AX = mybir.AxisListType.X
            ADD = mybir.AluOpType.add
            MUL = mybir.AluOpType.mult

            def edge_group(i, g):
                """Gather + edge MLP + fused segsum for blocks of group g.
                Returns the SBUF gacc tile [128, cs, 193]."""
                cs = cs_g[g]
                b0 = g * NB_G
                tile_base = TPB * NB_G * g
                gacc = gaccp.tile([128, cs, 193], f32, tag="gacc")
                import concourse.bass as bass

                def gather_split(out_tile, src_ap, idx_sb, col0, n_idx,
                                 elem):
                    # dma_gather crashes the device above num_idxs=1024
                    off = 0
                    while off < n_idx:
                        n = min(1024, n_idx - off)
                        nc.gpsimd.dma_gather(
                            out_tile[:, off // 128:(off + n) // 128, :],
                            src_ap,
                            idx_sb[:, col0 + off // 16:col0 + (off + n) // 16],
                            n, n, elem)
                        off += n

                for c in range(NC_WIN):
                    T_c = T_cs[c]
                    kt = cs * T_c
                    t0 = tile_base + coff[c] * cs
                    col0 = t0 * 8
                    gt = edgew.tile([128, kt, PK], bf, tag=f"gt{c % 2}")
                    if GATHER_GT:
                        gather_split(
                            gt, pfull[i][c * W:min(c * W + 32768, NF), :],
                            tsb, col0, kt * 128, PK)
                    else:
                        for tt in range(kt):
                            nc.gpsimd.indirect_dma_start(
                                out=gt[:, tt, :], out_offset=None,
                                in_=pfull[i][:],
                                in_offset=bass.IndirectOffsetOnAxis(
                                    ap=tsb32[:, t0 + tt:t0 + tt + 1], axis=0))
                    p0t = edgew.tile([128, kt * 128], bf, tag="p0t")
                    nc.sync.dma_start(
                        out=p0t[:],
                        in_=P_in["p0"][:, t0 * 128:(t0 + kt) * 128])
                    p1tt = edgew.tile([128, kt * 128], bf, tag="p1tt")
                    nc.scalar.dma_start(
                        out=p1tt[:],
                        in_=P_in["p1t"][:, t0 * 128:(t0 + kt) * 128])
                    # A1[h] per tile via permutation matmul; pre = A1[h]+B1[t]
                    pre = edgew.tile([128, kt, D], bf, tag="pre")
                    for t8 in range(0, kt, 8):
                        kk = min(8, kt - t8)
                        atp = psp.tile([128, 8, D], f32, tag="atp")
                        for tl in range(kk):
                            tt = t8 + tl
                            nc.tensor.matmul(
                                out=atp[:, tl, :],
                                lhsT=p1tt[:, tt * 128:(tt + 1) * 128],
                                rhs=a1sb[:, b0 + tt // T_c, :],
                                start=True, stop=True)
                        nc.vector.tensor_tensor(
                            out=pre[:, t8:t8 + kk, :], in0=atp[:, 0:kk, :],
                            in1=gt[:, t8:t8 + kk, 192:256], op=ADD)
                    rel = edgew.tile([128, kt, D], bf, tag="rel")
                    nc.scalar.activation(out=rel[:], in_=pre[:], func=Relu)
                    mr = edgew.tile([128, kt, D], f32, tag="mr")
                    nc.vector.tensor_tensor(out=mr[:], in0=rel[:],
                                            in1=w2sb[i][:, 0:kt, :], op=MUL)
                    lg = edgew.tile([128, kt], f32, tag="lg")
                    nc.vector.tensor_reduce(out=lg[:], in_=mr[:], axis=AX,
                                            op=ADD)
                    lg2 = edgew.tile([128, kt], f32, tag="lg2")
                    nc.vector.tensor_add(out=lg2[:], in0=lg[:],
                                         in1=egsb[i][:, t0:t0 + kt])
                    wv = edgew.tile([128, kt], f32, tag="wv")
                    nc.scalar.activation(out=wv[:], in_=lg2[:], func=Sigm,
                                         scale=inv_t,
                                         bias=float(b2v[i]) * inv_t)
                    wg = edgew.tile([128, kt], f32, tag="wg")
                    nc.vector.tensor_mul(out=wg[:], in0=wv[:],
                                         in1=rgsb[:, t0:t0 + kt])
                    # fused segsum rhs [e0|x2 | e1*wg | wg]
                    st = edgew.tile([128, kt, 193], bf, tag="st")
                    nc.scalar.activation(out=st[:, :, 0:128],
                                         in_=gt[:, :, 0:128], func=Copy)
                    nc.vector.tensor_tensor(
                        out=st[:, :, 128:192], in0=gt[:, :, 128:192],
                        in1=wg[:, :, None].to_broadcast([128, kt, D]),
                        op=MUL)
                    nc.vector.tensor_copy(out=st[:, :, 192:193],
                                          in_=wg[:, :, None])
                    for b in range(cs):
                        pacc = psaccp.tile([128, 193], f32, tag="pacc")
                        for jj in range(T_c):
                            tt = b * T_c + jj
                            nc.tensor.matmul(out=pacc[:],
                                             lhsT=p0t[:, tt * 128:(tt + 1) * 128],
                                             rhs=st[:, tt, :],
                                             start=(jj == 0),
                                             stop=(jj == T_c - 1))
                        if c == 0:
                            nc.scalar.activation(out=gacc[:, b, :],
                                                 in_=pacc[:], func=Copy)
                        else:
                            nc.vector.tensor_add(out=gacc[:, b, :],
                                                 in0=gacc[:, b, :],
                                                 in1=pacc[:])
                return gacc

            def node_group(i, g, gacc):
                """Update e/s tables for group g from gacc (i>0), then (i<L)
                compute A1/B1/x2 and write packed rows + a1d."""
                cs = cs_g[g]
                r0 = g * NB_G * 128
                rows = slice(r0, r0 + cs * 128)
                CF = cs * 128
                ets = []
                for kname, kd in (("e0", e0d), ("e1", e1d), ("e2", e2d)):
                    et = nodew.tile([128, cs, D], f32, tag=f"{kname}t")
                    nc.sync.dma_start(
                        out=et[:],
                        in_=kd[rows].rearrange("(c p) d -> p c d", p=128))
                    ets.append(et)
                e0t, e1t, e2t = ets
                if i > 0:
                    rsafe = nodew.tile([128, cs], f32, tag="rsafe")
                    nc.vector.tensor_scalar_max(
                        out=rsafe[:], in0=gacc[:, :, 192], scalar1=ROW_EPS)
                    dinv = nodew.tile([128, cs], f32, tag="dinv")
                    nc.vector.reciprocal(out=dinv[:], in_=rsafe[:])
                    g1s = nodew.tile([128, cs, D], f32, tag="g1s")
                    nc.vector.tensor_tensor(
                        out=g1s[:], in0=gacc[:, :, 128:192],
                        in1=dinv[:, :, None].to_broadcast([128, cs, D]),
                        op=MUL)
                    nc.vector.tensor_add(out=e1t[:], in0=e1t[:], in1=g1s[:])
                    nc.vector.tensor_add(out=e0t[:], in0=e0t[:],
                                         in1=gacc[:, :, 0:64])
                    nc.vector.tensor_add(out=e2t[:], in0=e2t[:],
                                         in1=gacc[:, :, 64:128])
                    for kd, et in ((e0d, e0t), (e1d, e1t), (e2d, e2t)):
                        nc.sync.dma_start(
                            out=kd[rows].rearrange("(c p) d -> p c d", p=128),
                            in_=et[:])
                    for kname, sd, et in (("s0", s0d, e0t), ("s1", s1d, e1t),
                                          ("s2", s2d, e2t)):
                        stl = nodew.tile([128, cs, D], f32, tag=f"{kname}t")
                        nc.sync.dma_start(
                            out=stl[:],
                            in_=sd[rows].rearrange("(c p) d -> p c d", p=128))
                        nc.vector.tensor_add(out=stl[:], in0=stl[:], in1=et[:])
                        nc.sync.dma_start(
                            out=sd[rows].rearrange("(c p) d -> p c d", p=128),
                            in_=stl[:])
                        if i == L:
                            k = int(kname[1])
                            nc.sync.dma_start(
                                out=out[k, rows].rearrange(
                                    "(c p) d -> p c d", p=128),
                                in_=stl[:])
                if i >= L:
                    return
                # ---- pack phase for layer i
                e1T = chunkw.tile([D, CF], f32, tag="e1T")
                if i == 0:
                    e2T = e1T          # e1 == e2 == emb at layer 0
                    pairs = ((e1t, e1T),)
                else:
                    e2T = chunkw.tile([D, CF], f32, tag="e2T")
                    pairs = ((e1t, e1T), (e2t, e2T))
                for q in range(cs):
                    for src, dstT in pairs:
                        pt = psp.tile([D, 128], f32, tag="ptr")
                        nc.tensor.transpose(
                            out=pt[:], in_=src[:, q, :], identity=ident[:])
                        nc.scalar.activation(
                            out=dstT[:, q * 128:(q + 1) * 128], in_=pt[:],
                            func=Copy)
                a1T = chunkw.tile([D, CF], f32, tag="a1T")
                b1T = chunkw.tile([D, CF], f32, tag="b1T")
                lgT = chunkw.tile([D, CF], f32, tag="lgT")
                pm = psp.tile([D, CF], f32, tag="pmm")
                nc.tensor.matmul(out=pm[:], lhsT=wt[("w1t", i)][:], rhs=e1T[:],
                                 start=True, stop=True)
                nc.scalar.activation(out=a1T[:], in_=pm[:], func=Ident,
                                     bias=wt[("b1", i)][:, 0:1])
                pm2 = psp.tile([D, CF], f32, tag="pmm")
                nc.tensor.matmul(out=pm2[:], lhsT=wt[("w1b", i)][:],
                                 rhs=e1T[:], start=True, stop=True)
                nc.scalar.activation(out=b1T[:], in_=pm2[:], func=Copy)
                pm3 = psp.tile([D, CF], f32, tag="pmm")
                nc.tensor.matmul(out=pm3[:], lhsT=wt[("ew1", i)][:],
                                 rhs=e2T[:], start=True, stop=True)
                hidT = chunkw.tile([D, CF], f32, tag="hidT")
                nc.scalar.activation(out=hidT[:], in_=pm3[:], func=Relu,
                                     bias=wt[("eb1", i)][:, 0:1])
                pm4 = psp.tile([D, CF], f32, tag="pmm")
                nc.tensor.matmul(out=pm4[:], lhsT=wt[("ew2", i)][:],
                                 rhs=hidT[:], start=True, stop=True)
                nc.scalar.activation(out=lgT[:], in_=pm4[:], func=Ident,
                                     bias=wt[("eb2", i)][:, 0:1])
                pk = nodew.tile([128, cs, PK], bf, tag="pk")
                for q in range(cs):
                    cols = slice(q * 128, (q + 1) * 128)
                    pa = psp.tile([128, D], f32, tag="ptr")
                    nc.tensor.transpose(out=pa[:], in_=a1T[:, cols],
                                        identity=ident[0:64, 0:64])
                    nc.scalar.activation(
                        out=a1sb[:, g * NB_G + q, :], in_=pa[:], func=Copy)
                    pb = psp.tile([128, D], f32, tag="ptr")
                    nc.tensor.transpose(out=pb[:], in_=b1T[:, cols],
                                        identity=ident[0:64, 0:64])
                    nc.vector.tensor_copy(out=pk[:, q, 192:256], in_=pb[:])
                    pl = psp.tile([128, D], f32, tag="ptr")
                    nc.tensor.transpose(out=pl[:], in_=lgT[:, cols],
                                        identity=ident[0:64, 0:64])
                    gmt = nodew.tile([128, D], f32, tag="gmt")
                    nc.sync.dma_start(out=gmt[:],
                                      in_=P_in["gum"][i, r0 + q * 128:
                                                      r0 + (q + 1) * 128, :])
                    lgn = nodew.tile([128, D], f32, tag="lgn")
                    nc.vector.tensor_add(out=lgn[:], in0=pl[:], in1=gmt[:])
                    gate = nodew.tile([128, D], f32, tag="gate")
                    nc.scalar.activation(out=gate[:], in_=lgn[:], func=Sigm,
                                         scale=inv_t)
                    nc.vector.tensor_mul(out=pk[:, q, 64:128], in0=gate[:],
                                         in1=e2t[:, q, :])
                    nc.vector.tensor_copy(out=pk[:, q, 0:64],
                                          in_=e0t[:, q, :])
                    nc.vector.tensor_copy(out=pk[:, q, 128:192],
                                          in_=e1t[:, q, :])
                kcc, koff = g_to_cc[g]
                nc.sync.dma_start(
                    out=pshard[kcc][koff:koff + cs * 128].rearrange(
                        "(c p) d -> p c d", p=128),
                    in_=pk[:])

            g_to_cc = {}
            for kcc, gs in enumerate(cc_groups):
                off = 0
                for g in gs:
                    g_to_cc[g] = (kcc, off)
                    off += cs_g[g] * 128

            def fire_collective(i, kcc):
                b0c = NCC * cc_base[kcc]
                nc.gpsimd.collective_compute(
                    "AllGather", mybir.AluOpType.bypass,
                    replica_groups=rg_all,
                    ins=[pshard[kcc][:]],
                    outs=[pfull[i][b0c:b0c + NCC * cc_rows[kcc], :]])

            cc_last = {gs[-1]: kcc for kcc, gs in enumerate(cc_groups)}

            # layer-0 node pass (pack only)
            for g in range(n_grp):
                node_group(0, g, None)
                if g in cc_last:
                    fire_collective(0, cc_last[g])
            # main interleaved loop
            for i in range(L):
                for g in range(n_grp):
                    gacc = edge_group(i, g)
                    node_group(i + 1, g, gacc)
                    if i + 1 < L and g in cc_last:
                        fire_collective(i + 1, cc_last[g])

    if not nc.is_finalized():
        nc.finalize()
    return nc


def _setup(inputs, ncores=8):
    """Host prep + program build + per-core input maps."""
    pc = _prep(inputs, ncores)
    D = pc["D"]
    eW1 = np.asarray(inputs["edge_W1"]).astype(np.float32)
    eW2 = np.asarray(inputs["edge_W2"]).astype(np.float32)
    cfg = dict(nb=pc["nb"], T_cs=pc["T_cs"], TPB=pc["TPB"], L=pc["L"],
               ncores=ncores, D=D,
               W=pc["W"], tiles_total=pc["tiles_total"], n_grp=pc["n_grp"],
               cs_g=pc["cs_g"], cc_groups=pc["cc_groups"],
               cc_rows=pc["cc_rows"], cc_base=pc["cc_base"],
               b2=[float(x) for x in np.asarray(inputs["edge_b2"]).ravel()],
               inv_t=1.0)
    nc = build_program(cfg)
    KT_MAX = NB_G * max(pc["T_cs"])
    w2t = np.broadcast_to(
        np.tile(eW2[:, :, 0], (1, KT_MAX))[:, None, :],
        (eW2.shape[0], 128, KT_MAX * eW2.shape[1])).copy()
    shared = {
        "w1t": np.ascontiguousarray(eW1[:, :D, :]),
        "w1b": np.ascontiguousarray(eW1[:, D:, :]),
        "b1": np.asarray(inputs["edge_b1"]).astype(np.float32),
        "w2": w2t,
        "ew1": np.asarray(inputs["emb_W1"]).astype(np.float32),
        "ew2": np.asarray(inputs["emb_W2"]).astype(np.float32),
        "eb1": np.asarray(inputs["emb_b1"]).astype(np.float32),
        "eb2": np.asarray(inputs["emb_b2"]).astype(np.float32),
    }
    in_maps = []
    for c in range(ncores):
        m = {"emb": pc["embc"][c], "gum": pc["gumc"][c],
             "p0": pc["p0"][c], "tid16": pc["tid16"][c],
             "tid32": pc["tid32"][c], "p1t": pc["p1t"][c],
             "egum": pc["egc"][c], "rgs": pc["rg"][c]}
        m.update(shared)
        in_maps.append(m)
    return nc, in_maps, pc


def kernel(**inputs) -> np.ndarray:
    from concourse.bass_utils import run_bass_kernel_spmd

    NCC = 8
    nc, in_maps, pc = _setup(inputs, NCC)
    RS, N, D = pc["RS"], pc["N"], pc["D"]
    res = run_bass_kernel_spmd(nc, in_maps, list(range(NCC)))
    full = np.empty((3, N, D), np.float32)
    for c in range(NCC):
        full[:, c * RS:(c + 1) * RS] = res.results[c]["out"][:, :RS]
    return full
